# revision 18
# baseline (speedup 1.0000x reference)
"""MAGAT GNN message-passing kernel for 8 Trainium2 NeuronCores.

Math: the reference applies Sinkhorn-Knopp to adj0 but only uses the result
via `adj > 0`, and Sinkhorn preserves the zero/positive pattern exactly in
fp32. The input adj0 is uniform [0,1) so all but a handful (~9 of 67M) of
entries are positive -- the softmax mask is essentially all-ones. The device
therefore computes UNMASKED attention, which needs no adjacency data at all:

  p[i,j] = exp(leaky_relu(es_i + ed_j))
         = exp(es_i) * max(eB_j, r_i * eb_j),   r_i = exp(-0.8 es_i)
  eB_j = exp(ed_j), eb_j = exp(0.2 ed_j)

and since h' = num/den, the exp(es_i) row factor cancels. So each [128,1024]
attention tile is ONE 4x-mode tensor_scalar op on the Vector engine:
  p' = (rb * eb_j) max eB_j        (rb = r broadcast, per-partition scalars)
followed by 8 matmuls against [Wh | 1] accumulating numerator and row-sum
into PSUM (4 double-width accumulators x 2 ipass buffers = all 8 banks).
No transcendentals, no adjacency DMA, no mask multiply in the steady state;
the whole O(N^2 D) message passing runs on device at the PE stream rate.

Host precomputes the linear projections (Wh = x0@W packed with a ones
column, plus the 1-D gate vectors exp(-0.8 es), exp(ed), exp(0.2 ed)) --
O(N F D) numpy work shipped as ~2.5 MiB per core. Rows whose mask has a
zero are recomputed exactly on host and patched -- exact for any input.
Sharding: 8 cores = 4 heads x 2 row-halves, x0 rolled per core.
"""

import numpy as np
import ml_dtypes
from contextlib import ExitStack

import concourse.bacc as bacc
import concourse.mybir as mybir
import concourse.tile as tile
from concourse.bass_utils import run_bass_kernel_spmd

F32 = mybir.dt.float32
BF16 = mybir.dt.bfloat16
AF = mybir.ActivationFunctionType
OP = mybir.AluOpType

N, F, H, D = 4096, 128, 4, 128
NH = N // 2          # own rows per core
NC = N // 128        # 32 j-chunks
IPASS = 2            # i splits
IW = NH // IPASS     # 1024 i per pass
ALPHA = 0.2
DQ = D + 1           # 129: [Wh | 1]

_cache = {}


def _build():
    nc = bacc.Bacc("TRN2", target_bir_lowering=False, debug=False)
    whpD = nc.dram_tensor("whp", [128, NC * DQ], BF16, kind="ExternalInput").ap()
    rbD = nc.dram_tensor("rb", [128, NH], BF16, kind="ExternalInput").ap()
    eBcD = nc.dram_tensor("eBc", [128, NC], F32, kind="ExternalInput").ap()
    ebcD = nc.dram_tensor("ebc", [128, NC], F32, kind="ExternalInput").ap()
    x0oD = nc.dram_tensor("x0o", [128, (NH // 128) * F], F32, kind="ExternalInput").ap()
    out = nc.dram_tensor("out", [128, (NH // 128) * D], F32, kind="ExternalOutput").ap()

    with tile.TileContext(nc) as tc, ExitStack() as ctx:
        const = ctx.enter_context(tc.tile_pool(name="const", bufs=1))

        whp = const.tile([128, NC * DQ], BF16)
        whp3 = whp[:].rearrange("p (c q) -> p c q", c=NC)
        rb = const.tile([128, NH], BF16)
        eB_col = const.tile([128, NC], F32)
        eb_col = const.tile([128, NC], F32)
        x03 = const.tile([128, (NH // 128) * F], F32)
        x033 = x03[:].rearrange("p (c f) -> p c f", c=NH // 128)

        # DMA: partition-group splits so descriptors spread across queues
        nc.sync.dma_start(eB_col[:], eBcD)
        nc.sync.dma_start(eb_col[:], ebcD)
        for pg in range(4):
            ps = slice(pg * 32, (pg + 1) * 32)
            nc.sync.dma_start(rb[ps, 0:IW], rbD[ps, 0:IW])
        for pg in range(4):
            ps = slice(pg * 32, (pg + 1) * 32)
            nc.sync.dma_start(whp[ps, :], whpD[ps, :])
        for pg in range(4):
            ps = slice(pg * 32, (pg + 1) * 32)
            nc.sync.dma_start(rb[ps, IW:NH], rbD[ps, IW:NH])
        for pg in range(4):
            ps = slice(pg * 32, (pg + 1) * 32)
            nc.sync.dma_start(x03[ps, :], x0oD[ps, :])

        # steady state: one 4x tensor_scalar + 8 matmuls per (ipass, jc)
        atp = ctx.enter_context(tc.tile_pool(name="atp", bufs=8))
        epil = ctx.enter_context(tc.tile_pool(name="epil", bufs=2))
        mpsum = ctx.enter_context(tc.tile_pool(name="mpsum", bufs=2, space="PSUM"))

        for ip in range(IPASS):
            iw = slice(ip * IW, (ip + 1) * IW)
            # 4 double-width accumulators: tag t holds m-tiles 2t, 2t+1
            pacc = [mpsum.tile([128, 2 * DQ], F32, tag=f"acc{t}", name=f"acc_{ip}_{t}")
                    for t in range(4)]
            for jc in range(NC):
                p = atp.tile([128, IW], BF16, tag="p")
                nc.vector.tensor_scalar(p[:], rb[:, iw],
                                        eb_col[:, jc:jc + 1],
                                        eB_col[:, jc:jc + 1],
                                        OP.mult, OP.max)
                for m in range(8):
                    nc.tensor.matmul(
                        pacc[m // 2][:, (m % 2) * DQ:(m % 2) * DQ + DQ],
                        lhsT=p[:, m * 128:(m + 1) * 128],
                        rhs=whp3[:, jc, :],
                        start=(jc == 0), stop=(jc == NC - 1))

            # epilogue: one group per accumulator (2 m-tiles each)
            for g in range(4):
                acc = pacc[g]
                rec = epil.tile([128, 2], F32, tag="rec", name=f"rec_{ip}_{g}")
                for k in range(2):
                    nc.vector.reciprocal(rec[:, k:k + 1],
                                         acc[:, k * DQ + D:k * DQ + D + 1])
                hb = epil.tile([128, 256], BF16, tag="hb", name=f"hb_{ip}_{g}")
                for k in range(2):
                    nc.scalar.activation(hb[:, k * 128:(k + 1) * 128],
                                         acc[:, k * DQ:k * DQ + D], AF.Copy,
                                         scale=rec[:, k:k + 1])
                # elu(x) = max(x, min(exp(x),1)-1)
                E1 = epil.tile([128, 256], BF16, tag="E1", name=f"E1_{ip}_{g}")
                for k in range(2):
                    nc.scalar.activation(E1[:, k * 128:(k + 1) * 128],
                                         acc[:, k * DQ:k * DQ + D], AF.Exp,
                                         scale=rec[:, k:k + 1])
                F1 = epil.tile([128, 256], BF16, tag="F1", name=f"F1_{ip}_{g}")
                nc.vector.tensor_scalar(F1[:], E1[:], 1.0, -1.0, OP.min, OP.add)
                el1 = epil.tile([128, 256], BF16, tag="el1", name=f"el1_{ip}_{g}")
                nc.vector.tensor_max(el1[:], F1[:], hb[:])
                # residual + second elu (f32)
                r = epil.tile([128, 256], F32, tag="r", name=f"r_{ip}_{g}")
                x0sl = x033[:, ip * 8 + g * 2:ip * 8 + (g + 1) * 2, :]
                nc.vector.tensor_add(
                    r[:], el1[:],
                    x0sl.rearrange("p k d -> p (k d)"))
                E2 = epil.tile([128, 256], F32, tag="E2", name=f"E2_{ip}_{g}")
                nc.scalar.activation(E2[:], r[:], AF.Exp)
                F2 = epil.tile([128, 256], F32, tag="F2", name=f"F2_{ip}_{g}")
                nc.vector.tensor_scalar(F2[:], E2[:], 1.0, -1.0, OP.min, OP.add)
                y = epil.tile([128, 256], F32, tag="y", name=f"y_{ip}_{g}")
                nc.vector.tensor_max(y[:], F2[:], r[:])
                q0 = ip * 8 + g * 2
                for pg in range(4):
                    ps = slice(pg * 32, (pg + 1) * 32)
                    nc.sync.dma_start(out[ps, q0 * D:(q0 + 2) * D], y[ps, :])

    nc.compile()
    return nc


def _get_nc():
    if "nc" not in _cache:
        _cache["nc"] = _build()
    return _cache["nc"]


def make_in_maps(x0, adj0, W, a_src, a_dst):
    """Per-core input dict (adj0 unused on device -- mask handled on host)."""
    bf = ml_dtypes.bfloat16
    maps = []
    for c in range(8):
        h, half = c // 2, c % 2
        i0 = half * NH
        xr = np.concatenate([x0[i0:], x0[:i0]], axis=0) if i0 else x0
        Wh = xr @ W[h]                            # [N, D] f32
        es = Wh[:NH] @ a_src[h]                   # [NH] f32
        ed = Wh @ a_dst[h]                        # [N]  f32
        whp = np.empty((NC, 128, DQ), np.float32)
        whp[:, :, :D] = Wh.reshape(NC, 128, D)
        whp[:, :, D] = 1.0
        # device layout [128, NC*DQ]: partition = row-in-chunk
        whp = np.ascontiguousarray(whp.transpose(1, 0, 2).reshape(128, NC * DQ))
        maps.append(dict(
            whp=whp.astype(bf),
            rb=np.ascontiguousarray(
                np.broadcast_to(np.exp(-0.8 * es)[None, :], (128, NH))).astype(bf),
            eBc=np.ascontiguousarray(
                np.exp(ed).reshape(NC, 128).T.astype(np.float32)),
            ebc=np.ascontiguousarray(
                np.exp(ALPHA * ed).reshape(NC, 128).T.astype(np.float32)),
            x0o=np.ascontiguousarray(
                xr[:NH].reshape(NH // 128, 128, F).transpose(1, 0, 2)
                .reshape(128, -1)),
        ))
    return maps


def _patch_masked_rows(x1, x0, adj0, W, a_src, a_dst):
    """Recompute exactly (float64) every row whose mask has a zero entry."""
    zer = np.argwhere(~(adj0 > 0))
    if len(zer) == 0:
        return
    x064 = x0.astype(np.float64)
    for h in np.unique(zer[:, 0]):
        Wh = x064 @ W[h].astype(np.float64)
        es = Wh @ a_src[h].astype(np.float64)
        ed = Wh @ a_dst[h].astype(np.float64)
        for i in np.unique(zer[zer[:, 0] == h][:, 1]):
            e = es[i] + ed
            e = np.where(e > 0, e, ALPHA * e)
            p = np.exp(e)
            p[~(adj0[h, i] > 0)] = 0.0
            att = p / p.sum()
            hp = att @ Wh
            hp = np.where(hp > 0, hp, np.exp(np.minimum(hp, 0)) - 1)
            r = hp + x064[i]
            y = np.where(r > 0, r, np.exp(np.minimum(r, 0)) - 1)
            x1[i, h * D:(h + 1) * D] = y.astype(np.float32)


def kernel(x0, adj0, W, a_src, a_dst):
    nc = _get_nc()
    res = run_bass_kernel_spmd(nc, make_in_maps(x0, adj0, W, a_src, a_dst),
                               core_ids=list(range(8))).results
    x1 = np.empty((N, H * D), np.float32)
    for c in range(8):
        h, half = c // 2, c % 2
        i0 = half * NH
        x1[i0:i0 + NH, h * D:(h + 1) * D] = (
            res[c]["out"].reshape(128, NH // 128, D)
            .transpose(1, 0, 2).reshape(NH, D))
    _patch_masked_rows(x1, x0, adj0, W, a_src, a_dst)
    return x1


# revision 19
# speedup vs baseline: 1.1652x; 1.1652x over previous
"""MAGAT GNN message-passing kernel for 8 Trainium2 NeuronCores.

Math: the reference applies Sinkhorn-Knopp to adj0 but only uses the result
via `adj > 0`, and Sinkhorn preserves the zero/positive pattern exactly in
fp32. The input adj0 is uniform [0,1) so all but a handful (~9 of 67M) of
entries are positive -- the softmax mask is essentially all-ones. The device
therefore computes UNMASKED attention, which needs no adjacency data at all:

  p[i,j] = exp(leaky_relu(es_i + ed_j))
         = exp(es_i) * max(eB_j, r_i * eb_j),   r_i = exp(-0.8 es_i)
  eB_j = exp(ed_j), eb_j = exp(0.2 ed_j)

and since h' = num/den, the exp(es_i) row factor cancels. So each [128,1024]
attention tile is ONE 4x-mode tensor_scalar op on the Vector engine:
  p' = (rb * eb_j) max eB_j        (rb = r broadcast, per-partition scalars)
followed by 8 matmuls against [Wh | 1] accumulating numerator and row-sum
into PSUM (4 double-width accumulators x 2 ipass buffers = all 8 banks).
No transcendentals, no adjacency DMA, no mask multiply in the steady state;
the whole O(N^2 D) message passing runs on device at the PE stream rate.

Host precomputes the linear projections (Wh = x0@W packed with a ones
column, plus the 1-D gate vectors exp(-0.8 es), exp(ed), exp(0.2 ed)) --
O(N F D) numpy work shipped as ~2.5 MiB per core. Rows whose mask has a
zero are recomputed exactly on host and patched -- exact for any input.
Sharding: 8 cores = 4 heads x 2 row-halves, x0 rolled per core.
"""

import numpy as np
import ml_dtypes
from contextlib import ExitStack

import concourse.bacc as bacc
import concourse.mybir as mybir
import concourse.tile as tile
from concourse.bass_utils import run_bass_kernel_spmd

F32 = mybir.dt.float32
BF16 = mybir.dt.bfloat16
AF = mybir.ActivationFunctionType
OP = mybir.AluOpType

N, F, H, D = 4096, 128, 4, 128
NH = N // 2          # own rows per core
NC = N // 128        # 32 j-chunks
IPASS = 2            # i splits
IW = NH // IPASS     # 1024 i per pass
ALPHA = 0.2
DQ = D + 1           # 129: [Wh | 1]

_cache = {}


def _build():
    nc = bacc.Bacc("TRN2", target_bir_lowering=False, debug=False)
    whpD = nc.dram_tensor("whp", [128, NC * DQ], BF16, kind="ExternalInput").ap()
    rbD = nc.dram_tensor("rb", [128, NH], BF16, kind="ExternalInput").ap()
    eBcD = nc.dram_tensor("eBc", [128, NC], F32, kind="ExternalInput").ap()
    ebcD = nc.dram_tensor("ebc", [128, NC], F32, kind="ExternalInput").ap()
    x0oD = nc.dram_tensor("x0o", [128, (NH // 128) * F], F32, kind="ExternalInput").ap()
    out = nc.dram_tensor("out", [128, (NH // 128) * D], F32, kind="ExternalOutput").ap()

    with tile.TileContext(nc) as tc, ExitStack() as ctx:
        const = ctx.enter_context(tc.tile_pool(name="const", bufs=1))

        whp = const.tile([128, NC * DQ], BF16)
        whp3 = whp[:].rearrange("p (c q) -> p c q", c=NC)
        rb = const.tile([128, NH], BF16)
        eB_col = const.tile([128, NC], F32)
        eb_col = const.tile([128, NC], F32)
        x03 = const.tile([128, (NH // 128) * F], F32)
        x033 = x03[:].rearrange("p (c f) -> p c f", c=NH // 128)

        # DMA order: small gates first, then whp chunks, rb halves, x0o last
        nc.sync.dma_start(eB_col[:], eBcD)
        nc.sync.dma_start(eb_col[:], ebcD)
        nc.sync.dma_start(rb[:, 0:IW], rbD[:, 0:IW])
        for g in range(4):
            sl = slice(g * 8 * DQ, (g + 1) * 8 * DQ)
            nc.sync.dma_start(whp[:, sl], whpD[:, sl])
        nc.sync.dma_start(rb[:, IW:NH], rbD[:, IW:NH])
        nc.sync.dma_start(x03[:], x0oD)

        # steady state: one 4x tensor_scalar + 8 matmuls per (ipass, jc)
        atp = ctx.enter_context(tc.tile_pool(name="atp", bufs=8))
        epil = ctx.enter_context(tc.tile_pool(name="epil", bufs=2))
        mpsum = ctx.enter_context(tc.tile_pool(name="mpsum", bufs=2, space="PSUM"))

        for ip in range(IPASS):
            iw = slice(ip * IW, (ip + 1) * IW)
            # 4 double-width accumulators: tag t holds m-tiles 2t, 2t+1
            pacc = [mpsum.tile([128, 2 * DQ], F32, tag=f"acc{t}", name=f"acc_{ip}_{t}")
                    for t in range(4)]
            for jc in range(NC):
                p = atp.tile([128, IW], BF16, tag="p")
                nc.vector.tensor_scalar(p[:], rb[:, iw],
                                        eb_col[:, jc:jc + 1],
                                        eB_col[:, jc:jc + 1],
                                        OP.mult, OP.max)
                for m in range(8):
                    nc.tensor.matmul(
                        pacc[m // 2][:, (m % 2) * DQ:(m % 2) * DQ + DQ],
                        lhsT=p[:, m * 128:(m + 1) * 128],
                        rhs=whp3[:, jc, :],
                        start=(jc == 0), stop=(jc == NC - 1))

            # epilogue: one group per accumulator (2 m-tiles each)
            for g in range(4):
                acc = pacc[g]
                rec = epil.tile([128, 2], F32, tag="rec", name=f"rec_{ip}_{g}")
                for k in range(2):
                    nc.vector.reciprocal(rec[:, k:k + 1],
                                         acc[:, k * DQ + D:k * DQ + D + 1])
                hb = epil.tile([128, 256], BF16, tag="hb", name=f"hb_{ip}_{g}")
                for k in range(2):
                    nc.scalar.activation(hb[:, k * 128:(k + 1) * 128],
                                         acc[:, k * DQ:k * DQ + D], AF.Copy,
                                         scale=rec[:, k:k + 1])
                # elu(x) = max(x, min(exp(x),1)-1)
                E1 = epil.tile([128, 256], BF16, tag="E1", name=f"E1_{ip}_{g}")
                for k in range(2):
                    nc.scalar.activation(E1[:, k * 128:(k + 1) * 128],
                                         acc[:, k * DQ:k * DQ + D], AF.Exp,
                                         scale=rec[:, k:k + 1])
                F1 = epil.tile([128, 256], BF16, tag="F1", name=f"F1_{ip}_{g}")
                nc.vector.tensor_scalar(F1[:], E1[:], 1.0, -1.0, OP.min, OP.add)
                el1 = epil.tile([128, 256], BF16, tag="el1", name=f"el1_{ip}_{g}")
                nc.vector.tensor_max(el1[:], F1[:], hb[:])
                # residual + second elu (f32)
                r = epil.tile([128, 256], F32, tag="r", name=f"r_{ip}_{g}")
                x0sl = x033[:, ip * 8 + g * 2:ip * 8 + (g + 1) * 2, :]
                nc.vector.tensor_add(
                    r[:], el1[:],
                    x0sl.rearrange("p k d -> p (k d)"))
                E2 = epil.tile([128, 256], F32, tag="E2", name=f"E2_{ip}_{g}")
                nc.scalar.activation(E2[:], r[:], AF.Exp)
                F2 = epil.tile([128, 256], F32, tag="F2", name=f"F2_{ip}_{g}")
                nc.vector.tensor_scalar(F2[:], E2[:], 1.0, -1.0, OP.min, OP.add)
                y = epil.tile([128, 256], F32, tag="y", name=f"y_{ip}_{g}")
                nc.vector.tensor_max(y[:], F2[:], r[:])
                q0 = ip * 8 + g * 2
                nc.sync.dma_start(out[:, q0 * D:(q0 + 2) * D], y[:])

    nc.compile()
    return nc


def _get_nc():
    if "nc" not in _cache:
        _cache["nc"] = _build()
    return _cache["nc"]


def make_in_maps(x0, adj0, W, a_src, a_dst):
    """Per-core input dict (adj0 unused on device -- mask handled on host)."""
    bf = ml_dtypes.bfloat16
    maps = []
    for c in range(8):
        h, half = c // 2, c % 2
        i0 = half * NH
        xr = np.concatenate([x0[i0:], x0[:i0]], axis=0) if i0 else x0
        Wh = xr @ W[h]                            # [N, D] f32
        es = Wh[:NH] @ a_src[h]                   # [NH] f32
        ed = Wh @ a_dst[h]                        # [N]  f32
        whp = np.empty((NC, 128, DQ), np.float32)
        whp[:, :, :D] = Wh.reshape(NC, 128, D)
        whp[:, :, D] = 1.0
        # device layout [128, NC*DQ]: partition = row-in-chunk
        whp = np.ascontiguousarray(whp.transpose(1, 0, 2).reshape(128, NC * DQ))
        maps.append(dict(
            whp=whp.astype(bf),
            rb=np.ascontiguousarray(
                np.broadcast_to(np.exp(-0.8 * es)[None, :], (128, NH))).astype(bf),
            eBc=np.ascontiguousarray(
                np.exp(ed).reshape(NC, 128).T.astype(np.float32)),
            ebc=np.ascontiguousarray(
                np.exp(ALPHA * ed).reshape(NC, 128).T.astype(np.float32)),
            x0o=np.ascontiguousarray(
                xr[:NH].reshape(NH // 128, 128, F).transpose(1, 0, 2)
                .reshape(128, -1)),
        ))
    return maps


def _patch_masked_rows(x1, x0, adj0, W, a_src, a_dst):
    """Recompute exactly (float64) every row whose mask has a zero entry."""
    zer = np.argwhere(~(adj0 > 0))
    if len(zer) == 0:
        return
    x064 = x0.astype(np.float64)
    for h in np.unique(zer[:, 0]):
        Wh = x064 @ W[h].astype(np.float64)
        es = Wh @ a_src[h].astype(np.float64)
        ed = Wh @ a_dst[h].astype(np.float64)
        for i in np.unique(zer[zer[:, 0] == h][:, 1]):
            e = es[i] + ed
            e = np.where(e > 0, e, ALPHA * e)
            p = np.exp(e)
            p[~(adj0[h, i] > 0)] = 0.0
            att = p / p.sum()
            hp = att @ Wh
            hp = np.where(hp > 0, hp, np.exp(np.minimum(hp, 0)) - 1)
            r = hp + x064[i]
            y = np.where(r > 0, r, np.exp(np.minimum(r, 0)) - 1)
            x1[i, h * D:(h + 1) * D] = y.astype(np.float32)


def kernel(x0, adj0, W, a_src, a_dst):
    nc = _get_nc()
    res = run_bass_kernel_spmd(nc, make_in_maps(x0, adj0, W, a_src, a_dst),
                               core_ids=list(range(8))).results
    x1 = np.empty((N, H * D), np.float32)
    for c in range(8):
        h, half = c // 2, c % 2
        i0 = half * NH
        x1[i0:i0 + NH, h * D:(h + 1) * D] = (
            res[c]["out"].reshape(128, NH // 128, D)
            .transpose(1, 0, 2).reshape(NH, D))
    _patch_masked_rows(x1, x0, adj0, W, a_src, a_dst)
    return x1
